# revision 52
# baseline (speedup 1.0000x reference)
"""Trainium2 Bass kernel for AttentiveGraphPooling (gnn_message_passing).

Strategy: shard the 4096 graphs across 8 cores (512 graphs each). batch is
sorted, so each core owns a contiguous node range covering whole graphs ->
pooling / gather / GRU are all core-local, no collectives needed.

Per core, graphs go in 4 blocks of 128. Each block's node features (bf16,
node-major) are DMA'd once and stay resident in SBUF for all three passes
(init mean-pool + 2 timesteps); the host also supplies a pre-transposed
feature-major copy that is streamed per timestep for the W1 matmuls. A
per-tile one-hot E (node x local-graph, iota/is_equal on the vector engine)
turns the segment mean-pool into a TensorEngine matmul; E^T (for the
graph-context gather) is formed on the PE during phase A and kept resident.

The gate MLP uses distributivity to stay node-major with no per-node add:
  h1[n] = relu(W1 @ (x_n + g_b(n)) + b1) = relu(x_n @ W1^T + GW1[b(n)])
with GW1 = G @ W1^T + b1 built once per block/timestep. The gate scalar
sigmoid(h1 . w2 + b2) is computed batched over 4 node tiles (one relu /
mult / reduce / sigmoid per 4 tiles). The gate never multiplies x: it is
fused into the one-hot instead (Eg = is_equal(iota,b) * gate), so the
weighted pooling is Eg.T @ x. The GRU runs per graph-block with biases
folded in via K=1 matmuls.
"""

import os
import sys

import numpy as np

sys.path.insert(0, "/opt/trn_rl_repo")

H = 256
NBLK = 4  # graph blocks per core
GBLK = 128  # graphs per block
NUM_TIMESTEPS = 2
LCHUNK = 16  # node tiles per resident-load DMA
GB = 4  # gate batch (node tiles per batched gate pipeline)


def _build_program(NT, nblk=NBLK, P=128):
    """Build the single-core SPMD Bass program. NT = node tiles per block.
    P = number of gate-w2 entries that are >= 0 after the host permutes the
    hidden dim positives-first and folds |w2| into W1's rows: the gate
    pre-activation is then sum(relu(h1)) - 2*sum(relu(h1)[P:])."""
    from contextlib import ExitStack

    import concourse.bass as bass
    import concourse.tile as tile
    from concourse import bacc, mybir

    fp32 = mybir.dt.float32
    bf16 = mybir.dt.bfloat16

    NTP = NT * 128  # padded nodes per block

    nc = bacc.Bacc("TRN2", target_bir_lowering=False, debug=False)

    # ---- DRAM parameters (per-core inputs) ----
    x_d = nc.dram_tensor("xk", [nblk * NTP, H], bf16, kind="ExternalInput")
    xt_d = nc.dram_tensor("xkT", [nblk, 2, 128, NTP], bf16, kind="ExternalInput")
    bcols_d = nc.dram_tensor("bcols", [nblk, 128, NT], fp32, kind="ExternalInput")
    invc_d = nc.dram_tensor("invc", [nblk, GBLK, 1], fp32, kind="ExternalInput")
    w1t_d = nc.dram_tensor("w1t", [2, 128, H], bf16, kind="ExternalInput")
    b1r_d = nc.dram_tensor("b1row", [1, H], fp32, kind="ExternalInput")

    b2c_d = nc.dram_tensor("b2col", [128, 1], fp32, kind="ExternalInput")
    wih_d = nc.dram_tensor("wih_t", [2, 128, 3 * H], bf16, kind="ExternalInput")
    whh_d = nc.dram_tensor("whh_t", [2, 128, 3 * H], bf16, kind="ExternalInput")
    brz_d = nc.dram_tensor("bsum_rz", [1, 2 * H], fp32, kind="ExternalInput")
    bin_d = nc.dram_tensor("bihn", [1, H], fp32, kind="ExternalInput")
    bhn_d = nc.dram_tensor("bhhn", [1, H], fp32, kind="ExternalInput")
    iota_d = nc.dram_tensor("iota_row", [128, 128], bf16, kind="ExternalInput")
    eye_d = nc.dram_tensor("eye128", [128, 128], fp32, kind="ExternalInput")
    eyeb_d = nc.dram_tensor("eye128b", [128, 128], bf16, kind="ExternalInput")
    out_d = nc.dram_tensor("out", [nblk * GBLK, H], fp32, kind="ExternalOutput")

    with tile.TileContext(nc) as tc, ExitStack() as ctx:
        ep = ctx.enter_context  # shorthand

        const = ep(tc.tile_pool(name="const", bufs=1))
        xres = ep(tc.tile_pool(name="xres", bufs=3))
        etres = ep(tc.tile_pool(name="etres", bufs=1))
        xtstr = ep(tc.tile_pool(name="xtstr", bufs=3))
        bpool = ep(tc.tile_pool(name="bcols", bufs=2))
        epool = ep(tc.tile_pool(name="eoh", bufs=8))
        trsh = ep(tc.tile_pool(name="trsh", bufs=2))
        gtsb = ep(tc.tile_pool(name="gtsb", bufs=3))
        gsb = ep(tc.tile_pool(name="gsb", bufs=3))
        smallsb = ep(tc.tile_pool(name="smallsb", bufs=2))

        ps_pool = ep(tc.tile_pool(name="pspool", bufs=2, space="PSUM"))
        ps_h1 = ep(tc.tile_pool(name="psh1", bufs=3, space="PSUM"))
        ps_et = ps_h1  # share the slots: pset users run while h1 batches idle

        # ---- load constants ----
        def cload(shape, src, tag, dt=fp32):
            t = const.tile(shape, dt, tag=tag)
            nc.sync.dma_start(t[:], src)
            return t

        iota_row = cload([128, 128], iota_d[:], "c_iota", bf16)
        eye = cload([128, 128], eye_d[:], "c_eye")
        eyeb = cload([128, 128], eyeb_d[:], "c_eyeb", bf16)
        w1t = [cload([128, H], w1t_d[k], f"c_w1t{k}", bf16) for k in range(2)]
        b1row = cload([1, H], b1r_d[:], "c_b1r")

        b2col = cload([128, 1], b2c_d[:], "c_b2c")
        wih = [cload([128, 3 * H], wih_d[k], f"c_wih{k}", bf16) for k in range(2)]
        whh = [cload([128, 3 * H], whh_d[k], f"c_whh{k}", bf16) for k in range(2)]
        brz = cload([1, 2 * H], brz_d[:], "c_brz")
        bin_ = cload([1, H], bin_d[:], "c_bin")
        bhn = cload([1, H], bhn_d[:], "c_bhn")
        invc = [cload([GBLK, 1], invc_d[j], f"c_invc{j}") for j in range(nblk)]
        ones_row = const.tile([1, 128], fp32)
        nc.vector.memset(ones_row[:], 1.0)

        def fm_copy(g_ap, pool, tag, dt):
            """(128,256) graph-major -> feature-major (128,2,128) via PE."""
            gf = pool.tile([128, 2, GBLK], dt, tag=tag)
            for ki in range(2):
                tp = ps_et.tile([128, 128], fp32, tag="psh1")
                nc.tensor.matmul(tp[:], g_ap[:, ki * 128 : (ki + 1) * 128], eye[:],
                                 is_transpose=True, start=True, stop=True)
                nc.scalar.copy(gf[:, ki, :], tp[:])
            return gf

        NH = NT // 2  # resident x split in halves so the next block's first
        # half can load while this block's second half is still in use

        for j in range(nblk):
            # bcols first: it unblocks all one-hot builds (E compares + E^T
            # transposes), giving the PE work while the big x DMAs stream in
            bt = bpool.tile([128, NT], fp32, tag="bcols")
            nc.sync.dma_start(bt[:], bcols_d[j])

            # ---- resident x (node-major) for this block, two half tiles ----
            xhalves = []
            for h0 in (0, NH):
                xh = xres.tile([128, NH, H], bf16, tag="xres")
                for c0 in range(0, NH, LCHUNK):
                    cn = min(LCHUNK, NH - c0)
                    base = j * NTP + (h0 + c0) * 128
                    src = x_d[base : base + cn * 128, :].rearrange(
                        "(c p) h -> p c h", p=128
                    )
                    nc.sync.dma_start(xh[:, c0 : c0 + cn, :], src)
                xhalves.append(xh)

            def xat(t):
                return xhalves[t // NH][:, t % NH, :]

            etj = etres.tile([128, NT, 128], bf16, tag="etres")

            # ---- phase A: initial mean pool + resident E^T build ----
            pooled = ps_pool.tile([GBLK, H], fp32, tag="pspool")
            for t in range(NT):
                e = epool.tile([128, 128], bf16, tag="eoh")
                nc.vector.tensor_scalar(
                    e[:], iota_row[:], bt[:, t : t + 1], None,
                    op0=mybir.AluOpType.is_equal,
                )
                nc.tensor.matmul(
                    pooled[:], e[:], xat(t), start=(t == 0), stop=(t == NT - 1),
                    skip_group_check=True,
                )
                tp = ps_et.tile([128, 128], bf16, tag="psh1")
                nc.tensor.matmul(tp[:], e[:], eyeb[:], is_transpose=True,
                                 start=True, stop=True)
                nc.scalar.copy(etj[:, t, :], tp[:])
            g_gm = gsb.tile([GBLK, H], fp32, tag="gsb")
            nc.vector.tensor_scalar(
                g_gm[:], pooled[:], invc[j][:], None, op0=mybir.AluOpType.mult
            )
            g_fm = fm_copy(g_gm[:], gsb, "gfm", bf16)

            # ---- timesteps ----
            for ts in range(NUM_TIMESTEPS):
                # GW1 = G @ W1^T + b1  (graph-level, bf16)
                gw1p = ps_et.tile([GBLK, H], fp32, tag="psh1")
                for ki in range(2):
                    nc.tensor.matmul(gw1p[:], g_fm[:, ki, :], w1t[ki][:],
                                     start=(ki == 0), stop=False,
                                     skip_group_check=True)
                nc.tensor.matmul(gw1p[:], ones_row[:], b1row[:],
                                 start=False, stop=True, skip_group_check=True)
                gw1 = gsb.tile([GBLK, H], bf16, tag="gw1")
                nc.scalar.copy(gw1[:], gw1p[:])

                pooled = ps_pool.tile([GBLK, H], fp32, tag="pspool")
                for t0 in range(0, NT, GB):
                    # stream x^T chunks (feature-major)
                    xts = xtstr.tile([128, 2, GB, 128], bf16, tag="xtstr")
                    for ki in range(2):
                        src = xt_d[j, ki, :, t0 * 128 : (t0 + GB) * 128].rearrange(
                            "p (c n) -> p c n", n=128
                        )
                        nc.sync.dma_start(xts[:, ki, :, :], src)
                    # h1 for GB tiles into one 2-bank PSUM tile
                    h1p = ps_h1.tile([128, GB, H], fp32, tag="psh1")
                    for c in range(GB):
                        t = t0 + c
                        for ki in range(2):
                            nc.tensor.matmul(
                                h1p[:, c, :], xts[:, ki, c, :], w1t[ki][:],
                                start=(ki == 0), stop=False,
                            )
                        nc.tensor.matmul(h1p[:, c, :], etj[:, t, :], gw1[:],
                                         start=False, stop=True)
                    # batched gate: |w2| is folded into W1's rows with
                    # negative-w2 columns grouped last, so the dot product is
                    # sum(relu) - 2*sum(relu over the negative range), taken
                    # from accum_out of per-tile relu ops (PSUM reads stay
                    # within one bank).
                    gpT = gtsb.tile([128, GB], fp32, tag="gpT")
                    gpN = gtsb.tile([128, GB], fp32, tag="gpN")
                    for c in range(GB):
                        trash = trsh.tile([128, H], bf16, tag="trsh")
                        nc.scalar.activation(
                            trash[:], h1p[:, c, :],
                            mybir.ActivationFunctionType.Relu,
                            accum_out=gpT[:, c : c + 1],
                        )
                        trn = trsh.tile([128, H - P], bf16, tag="trn")
                        nc.vector.tensor_scalar(
                            trn[:], h1p[:, c, P:], 0.0, 0.0,
                            op0=mybir.AluOpType.max,
                            op1=mybir.AluOpType.add,
                            accum_out=gpN[:, c : c + 1],
                        )
                    gpre = gtsb.tile([128, GB], fp32, tag="gpre")
                    nc.vector.scalar_tensor_tensor(
                        gpre[:], gpN[:], -2.0, gpT[:],
                        op0=mybir.AluOpType.mult, op1=mybir.AluOpType.add,
                    )
                    gt = gtsb.tile([128, GB], fp32, tag="gtsb")
                    nc.scalar.activation(
                        gt[:], gpre[:], mybir.ActivationFunctionType.Sigmoid,
                        bias=b2col[:],
                    )
                    # Eg = onehot * gate ; pooled += Eg.T @ x
                    for c in range(GB):
                        t = t0 + c
                        eg = epool.tile([128, 128], bf16, tag="eoh")
                        nc.vector.tensor_scalar(
                            eg[:], iota_row[:], bt[:, t : t + 1], gt[:, c : c + 1],
                            op0=mybir.AluOpType.is_equal, op1=mybir.AluOpType.mult,
                        )
                        nc.tensor.matmul(
                            pooled[:], eg[:], xat(t),
                            start=(t == 0), stop=(t == NT - 1),
                            skip_group_check=True,
                        )
                ps = gsb.tile([GBLK, H], fp32, tag="poolsb")
                nc.vector.tensor_scalar(
                    ps[:], pooled[:], invc[j][:], None, op0=mybir.AluOpType.mult
                )
                pf = fm_copy(ps[:], gsb, "poolfm", bf16)

                # ---- GRU cell (graph-major) ----
                gf, h_old = g_fm, g_gm

                def gru_mm(psum, wi, wh, bias_row, bcol0, bn):
                    mms = []
                    if wi is not None:
                        mms += [(pf[:, ki, :], wi[ki][:, bcol0 : bcol0 + bn])
                                for ki in range(2)]
                    if wh is not None:
                        mms += [(gf[:, ki, :], wh[ki][:, bcol0 : bcol0 + bn])
                                for ki in range(2)]
                    for i, (lhsT, rhs) in enumerate(mms):
                        nc.tensor.matmul(
                            psum[:], lhsT, rhs, start=(i == 0), stop=False,
                            skip_group_check=True,
                        )
                    nc.tensor.matmul(
                        psum[:], ones_row[:], bias_row, start=False, stop=True,
                        skip_group_check=True,
                    )

                # rp/zp are sigmoid'd immediately so their PSUM slots free
                # early; all four GRU accumulators share the psh1 slots.
                rp = ps_h1.tile([GBLK, H], fp32, tag="psh1")
                gru_mm(rp, wih, whh, brz[:, 0:H], 0, H)
                r = smallsb.tile([GBLK, H], fp32, tag="gru_r")
                nc.scalar.activation(r[:], rp[:], mybir.ActivationFunctionType.Sigmoid)
                zp = ps_h1.tile([GBLK, H], fp32, tag="psh1")
                gru_mm(zp, wih, whh, brz[:, H : 2 * H], H, H)
                z = smallsb.tile([GBLK, H], fp32, tag="gru_z")
                nc.scalar.activation(z[:], zp[:], mybir.ActivationFunctionType.Sigmoid)
                inp_ = ps_h1.tile([GBLK, H], fp32, tag="psh1")
                gru_mm(inp_, wih, None, bin_[:], 2 * H, H)
                hnp = ps_h1.tile([GBLK, H], fp32, tag="psh1")
                gru_mm(hnp, None, whh, bhn[:], 2 * H, H)
                t1 = smallsb.tile([GBLK, H], fp32, tag="gru_s1")
                nc.vector.tensor_mul(t1[:], r[:], hnp[:])
                t2 = smallsb.tile([GBLK, H], fp32, tag="gru_s2")
                nc.vector.tensor_add(t2[:], t1[:], inp_[:])
                n = smallsb.tile([GBLK, H], fp32, tag="gru_n")
                nc.scalar.activation(n[:], t2[:], mybir.ActivationFunctionType.Tanh)
                t3 = smallsb.tile([GBLK, H], fp32, tag="gru_s1")
                nc.vector.tensor_sub(t3[:], h_old[:], n[:])
                t4 = smallsb.tile([GBLK, H], fp32, tag="gru_s2")
                nc.vector.tensor_mul(t4[:], z[:], t3[:])
                t5 = smallsb.tile([GBLK, H], fp32, tag="gru_s3")
                nc.vector.tensor_add(t5[:], n[:], t4[:])
                g_gm = gsb.tile([GBLK, H], fp32, tag="gsb")
                nc.scalar.activation(g_gm[:], t5[:],
                                     mybir.ActivationFunctionType.Relu)
                if ts < NUM_TIMESTEPS - 1:
                    g_fm = fm_copy(g_gm[:], gsb, "gfm", bf16)

            nc.sync.dma_start(out_d[j * GBLK : (j + 1) * GBLK, :], g_gm[:])

    nc.compile()
    return nc


def _prep_inputs(x, batch, counts, n_cores, nblk, NT=None):
    """Host-side shard + pad + layout. Returns (per_core, NT)."""
    import ml_dtypes

    G = n_cores * nblk * GBLK
    batch = np.asarray(batch).astype(np.int64)
    x = np.asarray(x, dtype=np.float32)

    edges = np.searchsorted(batch, np.arange(0, G + 1, GBLK))
    blk_cnt = np.diff(edges)
    if NT is None:
        NT = int(np.ceil(blk_cnt.max() / 128))
        NT = ((NT + GB - 1) // GB) * GB
        if NT % 2:
            NT += GB  # halves of the resident x tile must be equal
    NTP = NT * 128

    invc_all = (1.0 / np.maximum(counts, 1.0)).astype(np.float32)

    xb = x.astype(ml_dtypes.bfloat16)
    per_core = []
    for k in range(n_cores):
        xk = np.zeros((nblk * NTP, H), dtype=ml_dtypes.bfloat16)
        bcols = np.full((nblk, 128, NT), -1.0, dtype=np.float32)
        for j in range(nblk):
            bi = k * nblk + j
            lo, hi = edges[bi], edges[bi + 1]
            cnt = hi - lo
            xk[j * NTP : j * NTP + cnt] = xb[lo:hi]
            blp = np.full(NTP, -1.0, dtype=np.float32)
            blp[:cnt] = (batch[lo:hi] - (bi * GBLK)).astype(np.float32)
            bcols[j] = blp.reshape(NT, 128).T
        xkT = np.ascontiguousarray(
            xk.reshape(nblk, NTP, 2, 128).transpose(0, 2, 3, 1)
        )
        invc = invc_all[k * nblk * GBLK : (k + 1) * nblk * GBLK].reshape(
            nblk, GBLK, 1
        )
        per_core.append({"xk": xk, "xkT": xkT, "bcols": bcols,
                         "invc": np.ascontiguousarray(invc)})
    return per_core, NT


def _const_inputs(gate_w1, gate_b1, gate_w2, gate_b2, gru_w_ih, gru_w_hh,
                  gru_b_ih, gru_b_hh):
    import ml_dtypes

    f = np.float32
    bf = ml_dtypes.bfloat16
    c = {}
    w2 = np.asarray(gate_w2, f).reshape(H)
    perm = np.argsort(w2 < 0, kind="stable")  # w2 >= 0 columns first
    P = int((w2 >= 0).sum())
    w1p = np.asarray(gate_w1, f)[perm] * np.abs(w2[perm])[:, None]
    b1p = np.asarray(gate_b1, f)[perm] * np.abs(w2[perm])
    c["w1t"] = np.ascontiguousarray(w1p.T).reshape(2, 128, H).astype(bf)
    c["b1row"] = np.ascontiguousarray(b1p).reshape(1, H)
    c["b2col"] = np.full((128, 1), np.asarray(gate_b2, f).reshape(()), dtype=f)
    c["_P"] = P
    c["wih_t"] = np.ascontiguousarray(
        np.asarray(gru_w_ih, f).T).reshape(2, 128, 3 * H).astype(bf)
    c["whh_t"] = np.ascontiguousarray(
        np.asarray(gru_w_hh, f).T).reshape(2, 128, 3 * H).astype(bf)
    bih = np.asarray(gru_b_ih, f)
    bhh = np.asarray(gru_b_hh, f)
    c["bsum_rz"] = (bih[: 2 * H] + bhh[: 2 * H]).reshape(1, 2 * H)
    c["bihn"] = bih[2 * H :].reshape(1, H)
    c["bhhn"] = bhh[2 * H :].reshape(1, H)
    c["iota_row"] = np.tile(np.arange(128, dtype=f), (128, 1)).astype(bf)
    c["eye128"] = np.eye(128, dtype=f)
    c["eye128b"] = np.eye(128, dtype=f).astype(bf)
    return c


_CACHE = {}


def run(x, gate_w1, gate_b1, gate_w2, gate_b2, gru_w_ih, gru_w_hh, gru_b_ih,
        gru_b_hh, batch, num_graphs, n_cores=8, nblk=NBLK, trace=False,
        use_sim=False):
    from concourse.bass_utils import run_bass_kernel_spmd

    batch = np.asarray(batch).astype(np.int64)
    G = n_cores * nblk * GBLK
    counts = np.bincount(batch, minlength=G).astype(np.float32)
    per_core, NT = _prep_inputs(x, batch, counts, n_cores, nblk)
    consts = _const_inputs(gate_w1, gate_b1, gate_w2, gate_b2, gru_w_ih,
                           gru_w_hh, gru_b_ih, gru_b_hh)
    P = consts.pop("_P")
    assert 0 < P < H, f"degenerate gate_w2 sign split P={P}"
    in_maps = [{**consts, **pc} for pc in per_core]

    key = (NT, nblk, n_cores, P)
    if key not in _CACHE:
        _CACHE[key] = _build_program(NT, nblk=nblk, P=P)
    nc = _CACHE[key]

    if use_sim:
        from concourse.bass_interp import CoreSim

        outs = []
        for k in range(n_cores):
            sim = CoreSim(nc)
            for name, arr in in_maps[k].items():
                sim.tensor(name)[:] = arr
            sim.simulate()
            outs.append(np.array(sim.tensor("out")))
        return np.concatenate(outs, axis=0), None

    res = run_bass_kernel_spmd(nc, in_maps, core_ids=list(range(n_cores)),
                               trace=trace)
    out = np.concatenate([res.results[k]["out"] for k in range(n_cores)], axis=0)
    return out, res


def kernel(**inputs):
    out, _ = run(**inputs)
    return out


# revision 55
# speedup vs baseline: 1.3200x; 1.3200x over previous
"""Trainium2 Bass kernel for AttentiveGraphPooling (gnn_message_passing).

Strategy: shard the 4096 graphs across 8 cores (512 graphs each). batch is
sorted, so each core owns a contiguous node range covering whole graphs ->
pooling / gather / GRU are all core-local, no collectives needed.

Per core, graphs go in 4 blocks of 128. Each block's node features (bf16,
node-major) are DMA'd once and stay resident in SBUF for all three passes
(init mean-pool + 2 timesteps); the host also supplies a pre-transposed
feature-major copy that is streamed per timestep for the W1 matmuls. A
per-tile one-hot E (node x local-graph, iota/is_equal on the vector engine)
turns the segment mean-pool into a TensorEngine matmul; E^T (for the
graph-context gather) is formed on the PE during phase A and kept resident.

The gate MLP uses distributivity to stay node-major with no per-node add:
  h1[n] = relu(W1 @ (x_n + g_b(n)) + b1) = relu(x_n @ W1^T + GW1[b(n)])
with GW1 = G @ W1^T + b1 built once per block/timestep. The gate scalar
sigmoid(h1 . w2 + b2) is computed batched over 4 node tiles (one relu /
mult / reduce / sigmoid per 4 tiles). The gate never multiplies x: it is
fused into the one-hot instead (Eg = is_equal(iota,b) * gate), so the
weighted pooling is Eg.T @ x. The GRU runs per graph-block with biases
folded in via K=1 matmuls.
"""

import os
import sys

import numpy as np

sys.path.insert(0, "/opt/trn_rl_repo")

H = 256
NBLK = 4  # graph blocks per core
GBLK = 128  # graphs per block
NUM_TIMESTEPS = 2
LCHUNK = 16  # node tiles per resident-load DMA
GB = 4  # gate batch (node tiles per batched gate pipeline)


def _build_program(NT, nblk=NBLK):
    """Build the single-core SPMD Bass program. NT = node tiles per block."""
    from contextlib import ExitStack

    import concourse.bass as bass
    import concourse.tile as tile
    from concourse import bacc, mybir

    fp32 = mybir.dt.float32
    bf16 = mybir.dt.bfloat16

    NTP = NT * 128  # padded nodes per block

    nc = bacc.Bacc("TRN2", target_bir_lowering=False, debug=False)

    # ---- DRAM parameters (per-core inputs) ----
    x_d = nc.dram_tensor("xk", [nblk * NTP, H], bf16, kind="ExternalInput")
    xt_d = nc.dram_tensor("xkT", [nblk, 2, 128, NTP], bf16, kind="ExternalInput")
    bcols_d = nc.dram_tensor("bcols", [nblk, 128, NT], fp32, kind="ExternalInput")
    invc_d = nc.dram_tensor("invc", [nblk, GBLK, 1], fp32, kind="ExternalInput")
    w1t_d = nc.dram_tensor("w1t", [2, 128, H], bf16, kind="ExternalInput")
    b1r_d = nc.dram_tensor("b1row", [1, H], fp32, kind="ExternalInput")
    w2bc_d = nc.dram_tensor("w2bc", [128, GB, H], bf16, kind="ExternalInput")
    b2c_d = nc.dram_tensor("b2col", [128, 1], fp32, kind="ExternalInput")
    wih_d = nc.dram_tensor("wih_t", [2, 128, 3 * H], bf16, kind="ExternalInput")
    whh_d = nc.dram_tensor("whh_t", [2, 128, 3 * H], bf16, kind="ExternalInput")
    brz_d = nc.dram_tensor("bsum_rz", [1, 2 * H], fp32, kind="ExternalInput")
    bin_d = nc.dram_tensor("bihn", [1, H], fp32, kind="ExternalInput")
    bhn_d = nc.dram_tensor("bhhn", [1, H], fp32, kind="ExternalInput")
    iota_d = nc.dram_tensor("iota_row", [128, 128], bf16, kind="ExternalInput")
    eye_d = nc.dram_tensor("eye128", [128, 128], fp32, kind="ExternalInput")
    eyeb_d = nc.dram_tensor("eye128b", [128, 128], bf16, kind="ExternalInput")
    out_d = nc.dram_tensor("out", [nblk * GBLK, H], fp32, kind="ExternalOutput")

    with tile.TileContext(nc) as tc, ExitStack() as ctx:
        ep = ctx.enter_context  # shorthand

        const = ep(tc.tile_pool(name="const", bufs=1))
        xres = ep(tc.tile_pool(name="xres", bufs=3))
        etres = ep(tc.tile_pool(name="etres", bufs=1))
        xtstr = ep(tc.tile_pool(name="xtstr", bufs=4))
        bpool = ep(tc.tile_pool(name="bcols", bufs=2))
        epool = ep(tc.tile_pool(name="eoh", bufs=8))
        trsh = ep(tc.tile_pool(name="trsh", bufs=3))
        gtsb = ep(tc.tile_pool(name="gtsb", bufs=4))
        gsb = ep(tc.tile_pool(name="gsb", bufs=3))
        smallsb = ep(tc.tile_pool(name="smallsb", bufs=2))

        ps_pool = ep(tc.tile_pool(name="pspool", bufs=2, space="PSUM"))
        ps_h1 = ep(tc.tile_pool(name="psh1", bufs=3, space="PSUM"))
        ps_et = ps_h1  # share the slots: pset users run while h1 batches idle

        # ---- load constants ----
        def cload(shape, src, tag, dt=fp32):
            t = const.tile(shape, dt, tag=tag)
            nc.sync.dma_start(t[:], src)
            return t

        iota_row = cload([128, 128], iota_d[:], "c_iota", bf16)
        eye = cload([128, 128], eye_d[:], "c_eye")
        eyeb = cload([128, 128], eyeb_d[:], "c_eyeb", bf16)
        w1t = [cload([128, H], w1t_d[k], f"c_w1t{k}", bf16) for k in range(2)]
        b1row = cload([1, H], b1r_d[:], "c_b1r")
        w2bc = cload([128, GB, H], w2bc_d[:], "c_w2bc", bf16)
        b2col = cload([128, 1], b2c_d[:], "c_b2c")
        wih = [cload([128, 3 * H], wih_d[k], f"c_wih{k}", bf16) for k in range(2)]
        whh = [cload([128, 3 * H], whh_d[k], f"c_whh{k}", bf16) for k in range(2)]
        brz = cload([1, 2 * H], brz_d[:], "c_brz")
        bin_ = cload([1, H], bin_d[:], "c_bin")
        bhn = cload([1, H], bhn_d[:], "c_bhn")
        invc = [cload([GBLK, 1], invc_d[j], f"c_invc{j}") for j in range(nblk)]
        ones_row = const.tile([1, 128], fp32)
        nc.vector.memset(ones_row[:], 1.0)

        def fm_copy(g_ap, pool, tag, dt):
            """(128,256) graph-major -> feature-major (128,2,128) via PE."""
            gf = pool.tile([128, 2, GBLK], dt, tag=tag)
            for ki in range(2):
                tp = ps_et.tile([128, 128], fp32, tag="psh1")
                nc.tensor.matmul(tp[:], g_ap[:, ki * 128 : (ki + 1) * 128], eye[:],
                                 is_transpose=True, start=True, stop=True)
                nc.scalar.copy(gf[:, ki, :], tp[:])
            return gf

        NH = NT // 2  # resident x split in halves so the next block's first
        # half can load while this block's second half is still in use

        for j in range(nblk):
            # bcols first: it unblocks all one-hot builds (E compares + E^T
            # transposes), giving the PE work while the big x DMAs stream in
            bt = bpool.tile([128, NT], fp32, tag="bcols")
            nc.sync.dma_start(bt[:], bcols_d[j])

            # ---- resident x (node-major) for this block, two half tiles ----
            xhalves = []
            for h0 in (0, NH):
                xh = xres.tile([128, NH, H], bf16, tag="xres")
                for c0 in range(0, NH, LCHUNK):
                    cn = min(LCHUNK, NH - c0)
                    base = j * NTP + (h0 + c0) * 128
                    src = x_d[base : base + cn * 128, :].rearrange(
                        "(c p) h -> p c h", p=128
                    )
                    nc.sync.dma_start(xh[:, c0 : c0 + cn, :], src)
                xhalves.append(xh)

            def xat(t):
                return xhalves[t // NH][:, t % NH, :]

            etj = etres.tile([128, NT, 128], bf16, tag="etres")

            # ---- phase A: initial mean pool + resident E^T build ----
            pooled = ps_pool.tile([GBLK, H], fp32, tag="pspool")
            for t in range(NT):
                e = epool.tile([128, 128], bf16, tag="eoh")
                nc.vector.tensor_scalar(
                    e[:], iota_row[:], bt[:, t : t + 1], None,
                    op0=mybir.AluOpType.is_equal,
                )
                nc.tensor.matmul(
                    pooled[:], e[:], xat(t), start=(t == 0), stop=(t == NT - 1),
                    skip_group_check=True,
                )
                tp = ps_et.tile([128, 128], bf16, tag="psh1")
                nc.tensor.matmul(tp[:], e[:], eyeb[:], is_transpose=True,
                                 start=True, stop=True)
                nc.scalar.copy(etj[:, t, :], tp[:])
            g_gm = gsb.tile([GBLK, H], fp32, tag="gsb")
            nc.vector.tensor_scalar(
                g_gm[:], pooled[:], invc[j][:], None, op0=mybir.AluOpType.mult
            )
            g_fm = fm_copy(g_gm[:], gsb, "gfm", bf16)

            # ---- timesteps ----
            for ts in range(NUM_TIMESTEPS):
                # GW1 = G @ W1^T + b1  (graph-level, bf16)
                gw1p = ps_et.tile([GBLK, H], fp32, tag="psh1")
                for ki in range(2):
                    nc.tensor.matmul(gw1p[:], g_fm[:, ki, :], w1t[ki][:],
                                     start=(ki == 0), stop=False,
                                     skip_group_check=True)
                nc.tensor.matmul(gw1p[:], ones_row[:], b1row[:],
                                 start=False, stop=True, skip_group_check=True)
                gw1 = gsb.tile([GBLK, H], bf16, tag="gw1")
                nc.scalar.copy(gw1[:], gw1p[:])

                pooled = ps_pool.tile([GBLK, H], fp32, tag="pspool")
                for t0 in range(0, NT, GB):
                    # stream x^T chunks (feature-major)
                    xts = xtstr.tile([128, 2, GB, 128], bf16, tag="xtstr")
                    for ki in range(2):
                        src = xt_d[j, ki, :, t0 * 128 : (t0 + GB) * 128].rearrange(
                            "p (c n) -> p c n", n=128
                        )
                        nc.sync.dma_start(xts[:, ki, :, :], src)
                    # h1 for GB tiles into one 2-bank PSUM tile
                    h1p = ps_h1.tile([128, GB, H], fp32, tag="psh1")
                    for c in range(GB):
                        t = t0 + c
                        for ki in range(2):
                            nc.tensor.matmul(
                                h1p[:, c, :], xts[:, ki, c, :], w1t[ki][:],
                                start=(ki == 0), stop=False,
                            )
                        nc.tensor.matmul(h1p[:, c, :], etj[:, t, :], gw1[:],
                                         start=False, stop=True)
                    # batched gate: relu -> .w2 -> rowsum -> sigmoid
                    # (relu split so each PSUM read stays within one bank)
                    trash = trsh.tile([128, GB, H], bf16, tag="trsh")
                    hb = GB // 2
                    nc.scalar.activation(trash[:, :hb, :], h1p[:, :hb, :],
                                         mybir.ActivationFunctionType.Relu)
                    nc.scalar.activation(trash[:, hb:, :], h1p[:, hb:, :],
                                         mybir.ActivationFunctionType.Relu)
                    prod = trsh.tile([128, GB, H], bf16, tag="prod")
                    nc.vector.tensor_mul(prod[:], trash[:], w2bc[:])
                    gpre = gtsb.tile([128, GB, 1], fp32, tag="gpre")
                    nc.vector.reduce_sum(gpre[:], prod[:], mybir.AxisListType.X)
                    gt = gtsb.tile([128, GB], fp32, tag="gtsb")
                    nc.scalar.activation(
                        gt[:], gpre[:, :, 0], mybir.ActivationFunctionType.Sigmoid,
                        bias=b2col[:],
                    )
                    # Eg = onehot * gate ; pooled += Eg.T @ x
                    for c in range(GB):
                        t = t0 + c
                        eg = epool.tile([128, 128], bf16, tag="eoh")
                        nc.vector.tensor_scalar(
                            eg[:], iota_row[:], bt[:, t : t + 1], gt[:, c : c + 1],
                            op0=mybir.AluOpType.is_equal, op1=mybir.AluOpType.mult,
                        )
                        nc.tensor.matmul(
                            pooled[:], eg[:], xat(t),
                            start=(t == 0), stop=(t == NT - 1),
                            skip_group_check=True,
                        )
                ps = gsb.tile([GBLK, H], fp32, tag="poolsb")
                nc.vector.tensor_scalar(
                    ps[:], pooled[:], invc[j][:], None, op0=mybir.AluOpType.mult
                )
                pf = fm_copy(ps[:], gsb, "poolfm", bf16)

                # ---- GRU cell (graph-major) ----
                gf, h_old = g_fm, g_gm

                def gru_mm(psum, wi, wh, bias_row, bcol0, bn):
                    mms = []
                    if wi is not None:
                        mms += [(pf[:, ki, :], wi[ki][:, bcol0 : bcol0 + bn])
                                for ki in range(2)]
                    if wh is not None:
                        mms += [(gf[:, ki, :], wh[ki][:, bcol0 : bcol0 + bn])
                                for ki in range(2)]
                    for i, (lhsT, rhs) in enumerate(mms):
                        nc.tensor.matmul(
                            psum[:], lhsT, rhs, start=(i == 0), stop=False,
                            skip_group_check=True,
                        )
                    nc.tensor.matmul(
                        psum[:], ones_row[:], bias_row, start=False, stop=True,
                        skip_group_check=True,
                    )

                # rp/zp are sigmoid'd immediately so their PSUM slots free
                # early; all four GRU accumulators share the psh1 slots.
                rp = ps_h1.tile([GBLK, H], fp32, tag="psh1")
                gru_mm(rp, wih, whh, brz[:, 0:H], 0, H)
                r = smallsb.tile([GBLK, H], fp32, tag="gru_r")
                nc.scalar.activation(r[:], rp[:], mybir.ActivationFunctionType.Sigmoid)
                zp = ps_h1.tile([GBLK, H], fp32, tag="psh1")
                gru_mm(zp, wih, whh, brz[:, H : 2 * H], H, H)
                z = smallsb.tile([GBLK, H], fp32, tag="gru_z")
                nc.scalar.activation(z[:], zp[:], mybir.ActivationFunctionType.Sigmoid)
                inp_ = ps_h1.tile([GBLK, H], fp32, tag="psh1")
                gru_mm(inp_, wih, None, bin_[:], 2 * H, H)
                hnp = ps_h1.tile([GBLK, H], fp32, tag="psh1")
                gru_mm(hnp, None, whh, bhn[:], 2 * H, H)
                t1 = smallsb.tile([GBLK, H], fp32, tag="gru_s1")
                nc.vector.tensor_mul(t1[:], r[:], hnp[:])
                t2 = smallsb.tile([GBLK, H], fp32, tag="gru_s2")
                nc.vector.tensor_add(t2[:], t1[:], inp_[:])
                n = smallsb.tile([GBLK, H], fp32, tag="gru_n")
                nc.scalar.activation(n[:], t2[:], mybir.ActivationFunctionType.Tanh)
                t3 = smallsb.tile([GBLK, H], fp32, tag="gru_s1")
                nc.vector.tensor_sub(t3[:], h_old[:], n[:])
                t4 = smallsb.tile([GBLK, H], fp32, tag="gru_s2")
                nc.vector.tensor_mul(t4[:], z[:], t3[:])
                t5 = smallsb.tile([GBLK, H], fp32, tag="gru_s3")
                nc.vector.tensor_add(t5[:], n[:], t4[:])
                g_gm = gsb.tile([GBLK, H], fp32, tag="gsb")
                nc.scalar.activation(g_gm[:], t5[:],
                                     mybir.ActivationFunctionType.Relu)
                if ts < NUM_TIMESTEPS - 1:
                    g_fm = fm_copy(g_gm[:], gsb, "gfm", bf16)

            nc.sync.dma_start(out_d[j * GBLK : (j + 1) * GBLK, :], g_gm[:])

    nc.compile()
    return nc


def _prep_inputs(x, batch, counts, n_cores, nblk, NT=None):
    """Host-side shard + pad + layout. Returns (per_core, NT)."""
    import ml_dtypes

    G = n_cores * nblk * GBLK
    batch = np.asarray(batch).astype(np.int64)
    x = np.asarray(x, dtype=np.float32)

    edges = np.searchsorted(batch, np.arange(0, G + 1, GBLK))
    blk_cnt = np.diff(edges)
    if NT is None:
        NT = int(np.ceil(blk_cnt.max() / 128))
        NT = ((NT + GB - 1) // GB) * GB
        if NT % 2:
            NT += GB  # halves of the resident x tile must be equal
    NTP = NT * 128

    invc_all = (1.0 / np.maximum(counts, 1.0)).astype(np.float32)

    xb = x.astype(ml_dtypes.bfloat16)
    per_core = []
    for k in range(n_cores):
        xk = np.zeros((nblk * NTP, H), dtype=ml_dtypes.bfloat16)
        bcols = np.full((nblk, 128, NT), -1.0, dtype=np.float32)
        for j in range(nblk):
            bi = k * nblk + j
            lo, hi = edges[bi], edges[bi + 1]
            cnt = hi - lo
            xk[j * NTP : j * NTP + cnt] = xb[lo:hi]
            blp = np.full(NTP, -1.0, dtype=np.float32)
            blp[:cnt] = (batch[lo:hi] - (bi * GBLK)).astype(np.float32)
            bcols[j] = blp.reshape(NT, 128).T
        xkT = np.ascontiguousarray(
            xk.reshape(nblk, NTP, 2, 128).transpose(0, 2, 3, 1)
        )
        invc = invc_all[k * nblk * GBLK : (k + 1) * nblk * GBLK].reshape(
            nblk, GBLK, 1
        )
        per_core.append({"xk": xk, "xkT": xkT, "bcols": bcols,
                         "invc": np.ascontiguousarray(invc)})
    return per_core, NT


def _const_inputs(gate_w1, gate_b1, gate_w2, gate_b2, gru_w_ih, gru_w_hh,
                  gru_b_ih, gru_b_hh):
    import ml_dtypes

    f = np.float32
    bf = ml_dtypes.bfloat16
    c = {}
    c["w1t"] = np.ascontiguousarray(
        np.asarray(gate_w1, f).T.reshape(2, 128, H)).astype(bf)
    c["b1row"] = np.asarray(gate_b1, f).reshape(1, H)
    c["w2bc"] = np.tile(np.asarray(gate_w2, f).reshape(1, 1, H),
                        (128, GB, 1)).astype(bf)
    c["b2col"] = np.full((128, 1), np.asarray(gate_b2, f).reshape(()), dtype=f)
    c["wih_t"] = np.ascontiguousarray(
        np.asarray(gru_w_ih, f).T).reshape(2, 128, 3 * H).astype(bf)
    c["whh_t"] = np.ascontiguousarray(
        np.asarray(gru_w_hh, f).T).reshape(2, 128, 3 * H).astype(bf)
    bih = np.asarray(gru_b_ih, f)
    bhh = np.asarray(gru_b_hh, f)
    c["bsum_rz"] = (bih[: 2 * H] + bhh[: 2 * H]).reshape(1, 2 * H)
    c["bihn"] = bih[2 * H :].reshape(1, H)
    c["bhhn"] = bhh[2 * H :].reshape(1, H)
    c["iota_row"] = np.tile(np.arange(128, dtype=f), (128, 1)).astype(bf)
    c["eye128"] = np.eye(128, dtype=f)
    c["eye128b"] = np.eye(128, dtype=f).astype(bf)
    return c


_CACHE = {}


def run(x, gate_w1, gate_b1, gate_w2, gate_b2, gru_w_ih, gru_w_hh, gru_b_ih,
        gru_b_hh, batch, num_graphs, n_cores=8, nblk=NBLK, trace=False,
        use_sim=False):
    from concourse.bass_utils import run_bass_kernel_spmd

    batch = np.asarray(batch).astype(np.int64)
    G = n_cores * nblk * GBLK
    counts = np.bincount(batch, minlength=G).astype(np.float32)
    per_core, NT = _prep_inputs(x, batch, counts, n_cores, nblk)
    consts = _const_inputs(gate_w1, gate_b1, gate_w2, gate_b2, gru_w_ih,
                           gru_w_hh, gru_b_ih, gru_b_hh)
    in_maps = [{**consts, **pc} for pc in per_core]

    key = (NT, nblk, n_cores)
    if key not in _CACHE:
        _CACHE[key] = _build_program(NT, nblk=nblk)
    nc = _CACHE[key]

    if use_sim:
        from concourse.bass_interp import CoreSim

        outs = []
        for k in range(n_cores):
            sim = CoreSim(nc)
            for name, arr in in_maps[k].items():
                sim.tensor(name)[:] = arr
            sim.simulate()
            outs.append(np.array(sim.tensor("out")))
        return np.concatenate(outs, axis=0), None

    res = run_bass_kernel_spmd(nc, in_maps, core_ids=list(range(n_cores)),
                               trace=trace)
    out = np.concatenate([res.results[k]["out"] for k in range(n_cores)], axis=0)
    return out, res


def kernel(**inputs):
    out, _ = run(**inputs)
    return out
